# revision 41
# baseline (speedup 1.0000x reference)
"""Depth-weighted 3x3 conv (DepthConv) Trainium2 Bass kernel (V3, bf16).

Math (per batch element):
  sim[k, p] = exp(-|depth[p + off_k] - depth[p]|)   (9 taps)
  out[o, p] = sum_{c,k} W[o,c,k] * sim[k,p] * x[c, p + off_k] + bias[o]

Sharding: data-parallel over batch, one batch element per NeuronCore (8).

Layout (unpadded 160-wide rows):
  SBUF partitions = 64 channels x {top, bottom half-image}.
  Free dim = flat image, 84 rows x 160 cols: pos row 0 guard (x=0),
  row 1 = top: image row -1 (x=0) / bottom: row 79 (halo), rows 2..81 =
  out rows, row 82 = top: row 80 (halo) / bottom: row 160 (x=0), row 83
  guard.  Out pixel q in [320, 13120); addressing is flat (rows wrap);
  horizontal wrap-around taps are killed by a +50 bias on |ddiff| before
  exp (sim -> 0) at the wrapping column (col 159 for maps (0,1),(1,1);
  col 0 for (1,-1)); vertical edge taps are killed by x=0 guard rows.

Tap symmetry: only 4 sim maps (center==1).  Per map m:
  tap +m uses xm_m[q]   = x[q+off]*map_m[q]  (x via parity-aligned copy)
  tap -m uses prod_m[q] = x[q]*map_m[q], read by the conv at q - off_m.

Everything is shaped by two measured costs: (a) every dma_start burns
~0.7us of SERIAL descriptor-generation (DIRECT2D) on its issuing engine,
so dma_start count is minimized and spread over sync/scalar/gpsimd;
(b) DVE tensor_tensor is hard-capped at 2 elem/cycle (7-lane crossbar),
so two xm products per segment run on the Pool engine instead.

Similarity pipeline (fp32 until exp, all 128-partition chunked):
  d2f32 [2, FLATG] fp32 depth bounce in SBUF, loaded at sync-ring head.
  ts8c/d0c [128, 810]: 8 rows (4 maps x 2 halves) x 16 column chunks,
  built with 6 merged DMAs (maps ordered so offs 159/160/161 fuse into
  one strided read).  DVE sub -> ACT abs -> DVE +wrapmask(0/50) ->
  ACT exp(-t) -> bf16.  The chunk layout IS row-flat, so one DMA drops
  it into s8d [8, 13120] DRAM (broadcast source), plus a zero head.

Broadcast: replicated DRAM->SBUF DMA per (map, half, segment), spread
across the three DMA-capable engines.

Conv: both halves share weights, so block-diag [128, 128] weights give
K=128 matmuls covering both halves at once (288 matmuls total, FWL
eligible); per 800-px chunk all 9 taps accumulate into one [128, 1024]
PSUM tile (bank-split 512+288), taps-outer so consecutive matmuls share
lhsT.  ACT evacuates +bias -> bf16 staging -> flat contiguous stores.
"""

import functools
import os
import sys

import numpy as np

for _p in ("/opt/trn_rl_repo",):
    if os.path.isdir(_p) and _p not in sys.path:
        sys.path.insert(0, _p)

import concourse.bass as bass
import concourse.mybir as mybir
import concourse.tile as tile
from concourse import bacc
from concourse.bass_utils import run_bass_kernel_spmd

# ---------------------------------------------------------------- constants
B, C, H, W = 8, 64, 160, 160
O = 64
KK = 9
NROW = 84                  # pos rows incl guards
FLATG = NROW * W           # 13440
XLEN = FLATG + 16          # x tiles padded so q+off+800 reads stay in-bounds
Q0 = 2 * W                 # 320: first out pixel (pos row 2)
NOUT = 80 * W              # 12800 out pixels per half
Q1 = Q0 + NOUT             # 13120
SIM0 = W                   # sim defined on [160, 13120): pos rows 1..81
SIMN = Q1 - SIM0           # 12960 = 16 * 810
NCHUNK = 16
CH = SIMN // NCHUNK        # 810
SROW = SIM0 + SIMN         # 13120: s8d row pitch (q-coords, zero head)
NCORES = 8

NSEG = 8
SEGQ = NOUT // NSEG        # 1600 (10 rows)
WINLO = 162                # window lead: covers prod reads (max off 161, even)
WIN = WINLO + SEGQ         # 1762
NCH = 2
CHW = SEGQ // NCH          # 800
SUBS = (512, 288)          # matmul N splits at the fp32 PSUM bank boundary

# map order chosen so offs 159,160,161 are consecutive (merged build DMA)
MAPS = [(0, 1, 1), (1, -1, W - 1), (1, 0, W), (1, 1, W + 1)]
# wrap-kill columns per map index (additive +50 before exp)
MASK_COLS = ((0, W - 1), (1, 0), (3, W - 1))
POOL_XMS = ()   # Pool muls contend with DVE's SBUF port -- keep off
# broadcast issuing engine per (map, half): 3 sync, 3 scalar, 2 gpsimd
BCAST_ENG = {
    (0, 0): "sync", (0, 1): "scalar",
    (1, 0): "gpsimd", (1, 1): "sync",
    (2, 0): "scalar", (2, 1): "gpsimd",
    (3, 0): "sync", (3, 1): "scalar",
}

F32 = mybir.dt.float32
F16 = mybir.dt.bfloat16


def _tapidx(dh, dw):
    return (dh + 1) * 3 + (dw + 1)


def _build_program():
    nc = bacc.Bacc(None)
    x_d = nc.declare_dram_parameter("x", [C, H, W], F32, isOutput=False)
    d_d = nc.declare_dram_parameter("depth", [H, W], F32, isOutput=False)
    wt_d = nc.declare_dram_parameter(
        "wt", [2 * C, KK, 2 * O], F32, isOutput=False
    )
    mask_d = nc.declare_dram_parameter("wmask", [128, CH], F32, isOutput=False)
    b_d = nc.declare_dram_parameter("bias2", [2 * O], F32, isOutput=False)
    out_d = nc.declare_dram_parameter("out", [O, H, W], F16, isOutput=True)

    Exp = mybir.ActivationFunctionType.Exp
    Abs = mybir.ActivationFunctionType.Abs
    Ident = mybir.ActivationFunctionType.Identity

    with tile.TileContext(nc) as tc:
        with (
            tc.tile_pool(name="dramp", bufs=1, space="DRAM") as dramp,
            tc.tile_pool(name="singles", bufs=1) as singles,
            tc.tile_pool(name="simp", bufs=14) as simp,
            tc.tile_pool(name="prodp", bufs=6) as prodp,
            tc.tile_pool(name="xmp", bufs=6) as xmp,
            tc.tile_pool(name="x2op", bufs=9) as x2op,
            tc.tile_pool(name="stgp", bufs=2) as stgp,
            tc.tile_pool(name="cpsum", bufs=2, space="PSUM") as cpsum,
        ):
            x2e = singles.tile([128, XLEN], F16)
            d2f = singles.tile([2, FLATG], F16)
            ts8c = singles.tile([128, CH], F16)
            d0c = singles.tile([128, CH], F16)
            ts8b = singles.tile([128, CH], F16)
            maskt = singles.tile([128, CH], F16)
            wt = singles.tile([128, KK, 2 * O], F16)
            b2 = singles.tile([128, 1], F32)
            zrow = singles.tile([1, 1312], F16)

            # ---------------- memsets (DVE; depth guards first so the sim
            # chain unblocks earliest)
            nc.vector.memset(zrow[:], 0.0)
            nc.vector.memset(d2f[0:2, 0 : 2 * W], 0.0)
            nc.vector.memset(d2f[0:2, Q1:FLATG], 0.0)
            nc.vector.memset(x2e[0:64, 0 : 2 * W], 0.0)
            nc.vector.memset(x2e[0:64, Q1 + W :], 0.0)
            nc.vector.memset(x2e[64:128, 0:W], 0.0)
            nc.vector.memset(x2e[64:128, Q1:], 0.0)

            # ---------------- depth chain (bf16; casting loads are SWDGE)
            hp = tc.high_priority()
            hp.__enter__()
            nc.gpsimd.dma_start(out=d2f[0:1, 2 * W : Q1 + W], in_=d_d[0:81, :])
            nc.gpsimd.dma_start(out=d2f[1:2, W:Q1], in_=d_d[79:160, :])
            nc.gpsimd.dma_start(out=maskt[:], in_=mask_d[:])
            nc.sync.dma_start(
                out=b2[:], in_=b_d.rearrange("(p one) -> p one", one=1)
            )
            # compact chunked dp: partition 16*(m+4h)+c <- d2f[h, SIM0+c*CH+off]
            for m, (dh, dw, off) in enumerate(MAPS):
                for h in range(2):
                    r = m + 4 * h
                    sl = d2f[h : h + 1, :]
                    src = bass.AP(
                        tensor=sl.tensor,
                        offset=sl.offset + SIM0 + off,
                        ap=[list(sl.ap)[0], [CH, NCHUNK], [1, CH]],
                    )
                    eng = nc.sync if r % 2 == 0 else nc.scalar
                    eng.dma_start(out=ts8c[16 * r : 16 * r + 16, :], in_=src)
            # d0 (map-independent; one 16-part group per (m, h))
            for m in range(4):
                for h in range(2):
                    r = m + 4 * h
                    sl = d2f[h : h + 1, :]
                    src = bass.AP(
                        tensor=sl.tensor,
                        offset=sl.offset + SIM0,
                        ap=[list(sl.ap)[0], [CH, NCHUNK], [1, CH]],
                    )
                    eng = nc.scalar if r % 2 == 0 else nc.sync
                    eng.dma_start(out=d0c[16 * r : 16 * r + 16, :], in_=src)

            # sim = exp(-(|dp - d0| + wrapmask)), wrapmask in {0, 50}
            nc.vector.tensor_sub(ts8c[:], ts8c[:], d0c[:])
            nc.scalar.activation(out=ts8c[:], in_=ts8c[:], func=Abs)
            nc.vector.tensor_add(ts8c[:], ts8c[:], maskt[:])
            nc.scalar.activation(
                out=ts8b[:], in_=ts8c[:], func=Exp, scale=-1.0
            )

            # chunk layout is row-flat: one DMA -> s8d [8, SROW] in DRAM
            # (row r at r*SROW, q-coords, zero head covers q in [0, 160))
            s8d = dramp.tile([8, SROW], F16)
            s8f = s8d[:]
            zdst = bass.AP(
                tensor=s8f.tensor,
                offset=s8f.offset,
                ap=[[SROW, 8], [1, SIM0]],
            )
            nc.sync.dma_start(out=zdst, in_=zrow[0:1, 0 : 8 * SIM0])
            mdst = bass.AP(
                tensor=s8f.tensor,
                offset=s8f.offset + SIM0,
                ap=[[SROW, 8], [CH, NCHUNK], [1, CH]],
            )
            nc.sync.dma_start(out=mdst, in_=ts8b[:])
            hp.__exit__(None, None, None)

            # ---------------- x loads (contiguous, casting SWDGE) + parity
            # 3 chunks x 27 rows per half; parity x2o[p] = x2e[p+1] follows
            # each chunk so early segments can start.  Modeled late
            # (wait_until) so the scheduler keeps the sim chain ahead of
            # them in the DMA rings.
            nc.gpsimd.dma_start(out=wt[:], in_=wt_d[:])
            with tc.tile_wait_until(0.03):
                for ci in range(3):
                    r0 = 27 * ci
                    nc.gpsimd.dma_start(
                        out=x2e[0:64, 2 * W + r0 * W : 2 * W + (r0 + 27) * W],
                        in_=x_d[:, r0 : r0 + 27, :].rearrange(
                            "c r w -> c (r w)"
                        ),
                    )
                    nc.gpsimd.dma_start(
                        out=x2e[64:128, W + r0 * W : W + (r0 + 27) * W],
                        in_=x_d[:, 79 + r0 : 79 + r0 + 27, :].rearrange(
                            "c r w -> c (r w)"
                        ),
                    )

            engs = {"sync": nc.sync, "scalar": nc.scalar, "gpsimd": nc.gpsimd}

            # per-segment odd-parity x spans, pre-issued so the loop has no
            # serial DMA-gen inside segments: x2os[s][i] = x2e[qs+1+i]
            x2os_all = []
            with tc.tile_wait_until(0.035):
                for s in range(NSEG):
                    qs = Q0 + s * SEGQ
                    x2os = x2op.tile([128, SEGQ + 164], F16, tag="x2o")
                    x2os_all.append(x2os)
                    nc.gpsimd.dma_start(
                        out=x2os[:, 0 : SEGQ + 160],
                        in_=x2e[:, qs + 1 : qs + 1 + SEGQ + 160],
                    )

            # ---------------- main loop (broadcasts prefetched 2 segs
            # ahead so the replicated DMAs overlap compute)
            PF = 0

            def issue_bcasts(s):
                qs = Q0 + s * SEGQ
                wa = qs - WINLO
                sims = []
                for m, (dh, dw, off) in enumerate(MAPS):
                    sim_m = simp.tile([128, WIN], F16, tag="sim")
                    sims.append(sim_m)
                    for h in range(2):
                        r = m + 4 * h
                        src = bass.AP(
                            tensor=s8f.tensor,
                            offset=s8f.offset + r * SROW + wa,
                            ap=[[0, 64], [1, WIN]],
                        )
                        engs[BCAST_ENG[(m, h)]].dma_start(
                            out=sim_m[64 * h : 64 * h + 64, :], in_=src
                        )
                return sims

            sims_q = [issue_bcasts(s) for s in range(min(PF + 1, NSEG))]
            for s in range(NSEG):
                qs = Q0 + s * SEGQ
                wa = qs - WINLO
                sims = sims_q[s]
                if s + PF + 1 < NSEG:
                    sims_q.append(issue_bcasts(s + PF + 1))

                x2os = x2os_all[s]

                # tight prods: prod_m covers [qs-off-d, +SEGQ+2d)
                prods = []
                pbase = []
                for m, (dh, dw, off) in enumerate(MAPS):
                    d = off % 2
                    pw = SEGQ + 2 * d
                    pr = prodp.tile([128, SEGQ + 2], F16, tag="prod")
                    prods.append(pr)
                    pbase.append(d)
                    ps0 = qs - off - d
                    nc.vector.tensor_mul(
                        pr[:, 0:pw],
                        x2e[:, ps0 : ps0 + pw],
                        sims[m][:, ps0 - wa : ps0 - wa + pw],
                    )

                stg = stgp.tile([128, SEGQ], F16, tag="stg")
                for j in range(NCH):
                    q = qs + j * CHW
                    so = WINLO + j * CHW           # even
                    psum = cpsum.tile([128, 1024], F32, tag="cps")
                    xms = []
                    for m, (dh, dw, off) in enumerate(MAPS):
                        xm = xmp.tile([128, CHW], F16, tag="xm")
                        xms.append(xm)
                        if off % 2:
                            xsrc = x2os[
                                :, off - 1 + j * CHW : off - 1 + j * CHW + CHW
                            ]
                        else:
                            xsrc = x2e[:, q + off : q + off + CHW]
                        eng = (
                            nc.gpsimd if (m, j) in POOL_XMS else nc.vector
                        )
                        eng.tensor_mul(
                            xm[:], xsrc, sims[m][:, so : so + CHW]
                        )
                    # taps-outer (consecutive matmuls share lhsT for
                    # ldw-opt); center + prod taps first, xm taps last.
                    taps = [(_tapidx(0, 0), x2e, q)]
                    for m, (dh, dw, off) in enumerate(MAPS):
                        taps.append(
                            (_tapidx(-dh, -dw), prods[m], pbase[m] + j * CHW)
                        )
                    for m, (dh, dw, off) in enumerate(MAPS):
                        taps.append((_tapidx(dh, dw), xms[m], 0))
                    for ti, (widx, rsrc, roff) in enumerate(taps):
                        for half in range(2):
                            pl, ph = 64 * half, 64 * half + 64
                            o2 = 0
                            for nn2 in SUBS:
                                nc.tensor.matmul(
                                    psum[pl:ph, o2 : o2 + nn2],
                                    wt[pl:ph, widx, 64 * half : 64 * half + 64],
                                    rsrc[pl:ph, roff + o2 : roff + o2 + nn2],
                                    start=(ti == 0),
                                    stop=(ti == len(taps) - 1),
                                    skip_group_check=True,
                                )
                                o2 += nn2
                    nc.scalar.activation(
                        out=stg[:, j * CHW : (j + 1) * CHW],
                        in_=psum[:, 0:CHW],
                        func=Ident,
                        bias=b2[:],
                        scale=1.0,
                    )

                of = out_d.rearrange("c h w -> c (h w)")
                nc.scalar.dma_start(
                    out=of[:, s * SEGQ : (s + 1) * SEGQ], in_=stg[0:64, :]
                )
                nc.scalar.dma_start(
                    out=of[:, NOUT + s * SEGQ : NOUT + (s + 1) * SEGQ],
                    in_=stg[64:128, :],
                )

    return nc


@functools.lru_cache(maxsize=1)
def _get_program():
    return _build_program()


def make_in_maps(x, depth, weights, bias):
    w_ckt = weights.reshape(O, C, KK).transpose(1, 2, 0)  # [C, KK, O]
    wt = np.zeros((2 * C, KK, 2 * O), np.float32)
    wt[0:C, :, 0:O] = w_ckt
    wt[C:, :, O:] = w_ckt
    wt = np.ascontiguousarray(wt)
    b2 = np.concatenate([bias, bias]).astype(np.float32)
    # wrap-around kill mask (additive, pre-exp) in the chunked compact-sim
    # layout: partition p = 16*(m+4h)+c, elem e <-> pixel q = SIM0+c*CH+e
    wmask = np.zeros((128, CH), np.float32)
    cols = np.arange(CH)
    for c in range(NCHUNK):
        colmod = (SIM0 + c * CH + cols) % W
        for m, col in MASK_COLS:
            for h in range(2):
                wmask[16 * (m + 4 * h) + c, colmod == col] = 50.0
    base = {"wt": wt, "bias2": b2, "wmask": wmask}
    return [
        {
            "x": np.ascontiguousarray(x[i]).astype(np.float32),
            "depth": np.ascontiguousarray(depth[i, 0]).astype(np.float32),
            **base,
        }
        for i in range(x.shape[0])
    ]


def kernel(x, depth, weights, bias):
    nc = _get_program()
    if not nc.is_finalized():
        nc.finalize()
    in_maps = make_in_maps(x, depth, weights, bias)
    res = run_bass_kernel_spmd(nc, in_maps, list(range(NCORES)))
    out = np.stack([np.asarray(res.results[i]["out"]) for i in range(NCORES)])
    return out.astype(np.float32)


# revision 42
# speedup vs baseline: 1.0025x; 1.0025x over previous
"""Depth-weighted 3x3 conv (DepthConv) Trainium2 Bass kernel (V3, bf16).

Math (per batch element):
  sim[k, p] = exp(-|depth[p + off_k] - depth[p]|)   (9 taps)
  out[o, p] = sum_{c,k} W[o,c,k] * sim[k,p] * x[c, p + off_k] + bias[o]

Sharding: data-parallel over batch, one batch element per NeuronCore (8).

Layout (unpadded 160-wide rows):
  SBUF partitions = 64 channels x {top, bottom half-image}.
  Free dim = flat image, 84 rows x 160 cols: pos row 0 guard (x=0),
  row 1 = top: image row -1 (x=0) / bottom: row 79 (halo), rows 2..81 =
  out rows, row 82 = top: row 80 (halo) / bottom: row 160 (x=0), row 83
  guard.  Out pixel q in [320, 13120); addressing is flat (rows wrap);
  horizontal wrap-around taps are killed by a +50 bias on |ddiff| before
  exp (sim -> 0) at the wrapping column (col 159 for maps (0,1),(1,1);
  col 0 for (1,-1)); vertical edge taps are killed by x=0 guard rows.

Tap symmetry: only 4 sim maps (center==1).  Per map m:
  tap +m uses xm_m[q]   = x[q+off]*map_m[q]  (x via parity-aligned copy)
  tap -m uses prod_m[q] = x[q]*map_m[q], read by the conv at q - off_m.

Everything is shaped by two measured costs: (a) every dma_start burns
~0.7us of SERIAL descriptor-generation (DIRECT2D) on its issuing engine,
so dma_start count is minimized and spread over sync/scalar/gpsimd;
(b) DVE tensor_tensor is hard-capped at 2 elem/cycle (7-lane crossbar),
so two xm products per segment run on the Pool engine instead.

Similarity pipeline (fp32 until exp, all 128-partition chunked):
  d2f32 [2, FLATG] fp32 depth bounce in SBUF, loaded at sync-ring head.
  ts8c/d0c [128, 810]: 8 rows (4 maps x 2 halves) x 16 column chunks,
  built with 6 merged DMAs (maps ordered so offs 159/160/161 fuse into
  one strided read).  DVE sub -> ACT abs -> DVE +wrapmask(0/50) ->
  ACT exp(-t) -> bf16.  The chunk layout IS row-flat, so one DMA drops
  it into s8d [8, 13120] DRAM (broadcast source), plus a zero head.

Broadcast: replicated DRAM->SBUF DMA per (map, half, segment), spread
across the three DMA-capable engines.

Conv: both halves share weights, so block-diag [128, 128] weights give
K=128 matmuls covering both halves at once (288 matmuls total, FWL
eligible); per 800-px chunk all 9 taps accumulate into one [128, 1024]
PSUM tile (bank-split 512+288), taps-outer so consecutive matmuls share
lhsT.  ACT evacuates +bias -> bf16 staging -> flat contiguous stores.
"""

import functools
import os
import sys

import numpy as np

for _p in ("/opt/trn_rl_repo",):
    if os.path.isdir(_p) and _p not in sys.path:
        sys.path.insert(0, _p)

import concourse.bass as bass
import concourse.mybir as mybir
import concourse.tile as tile
from concourse import bacc
from concourse.bass_utils import run_bass_kernel_spmd

# ---------------------------------------------------------------- constants
B, C, H, W = 8, 64, 160, 160
O = 64
KK = 9
NROW = 84                  # pos rows incl guards
FLATG = NROW * W           # 13440
XLEN = FLATG + 16          # x tiles padded so q+off+800 reads stay in-bounds
Q0 = 2 * W                 # 320: first out pixel (pos row 2)
NOUT = 80 * W              # 12800 out pixels per half
Q1 = Q0 + NOUT             # 13120
SIM0 = W                   # sim defined on [160, 13120): pos rows 1..81
SIMN = Q1 - SIM0           # 12960 = 16 * 810
NCHUNK = 16
CH = SIMN // NCHUNK        # 810
SROW = SIM0 + SIMN         # 13120: s8d row pitch (q-coords, zero head)
NCORES = 8

NSEG = 8
SEGQ = NOUT // NSEG        # 1600 (10 rows)
WINLO = 162                # window lead: covers prod reads (max off 161, even)
WIN = WINLO + SEGQ         # 1762
NCH = 2
CHW = SEGQ // NCH          # 800
SUBS = (512, 288)          # matmul N splits at the fp32 PSUM bank boundary

# map order chosen so offs 159,160,161 are consecutive (merged build DMA)
MAPS = [(0, 1, 1), (1, -1, W - 1), (1, 0, W), (1, 1, W + 1)]
# wrap-kill columns per map index (additive +50 before exp)
MASK_COLS = ((0, W - 1), (1, 0), (3, W - 1))
POOL_XMS = ()   # Pool muls contend with DVE's SBUF port -- keep off
# broadcast issuing engine per (map, half): 3 sync, 3 scalar, 2 gpsimd
BCAST_ENG = {
    (0, 0): "sync", (0, 1): "scalar",
    (1, 0): "gpsimd", (1, 1): "sync",
    (2, 0): "scalar", (2, 1): "gpsimd",
    (3, 0): "sync", (3, 1): "scalar",
}

F32 = mybir.dt.float32
F16 = mybir.dt.bfloat16


def _tapidx(dh, dw):
    return (dh + 1) * 3 + (dw + 1)


def _build_program():
    nc = bacc.Bacc(None)
    x_d = nc.declare_dram_parameter("x", [C, H, W], F32, isOutput=False)
    d_d = nc.declare_dram_parameter("depth", [H, W], F32, isOutput=False)
    wt_d = nc.declare_dram_parameter(
        "wt", [2 * C, KK, 2 * O], F32, isOutput=False
    )
    mask_d = nc.declare_dram_parameter("wmask", [128, CH], F32, isOutput=False)
    b_d = nc.declare_dram_parameter("bias2", [2 * O], F32, isOutput=False)
    out_d = nc.declare_dram_parameter("out", [O, H, W], F16, isOutput=True)

    Exp = mybir.ActivationFunctionType.Exp
    Abs = mybir.ActivationFunctionType.Abs
    Ident = mybir.ActivationFunctionType.Identity

    with tile.TileContext(nc) as tc:
        with (
            tc.tile_pool(name="dramp", bufs=1, space="DRAM") as dramp,
            tc.tile_pool(name="singles", bufs=1) as singles,
            tc.tile_pool(name="simp", bufs=13) as simp,
            tc.tile_pool(name="prodp", bufs=8) as prodp,
            tc.tile_pool(name="xmp", bufs=6) as xmp,
            tc.tile_pool(name="x2op", bufs=9) as x2op,
            tc.tile_pool(name="stgp", bufs=2) as stgp,
            tc.tile_pool(name="cpsum", bufs=2, space="PSUM") as cpsum,
        ):
            x2e = singles.tile([128, XLEN], F16)
            d2f = singles.tile([2, FLATG], F16)
            ts8c = singles.tile([128, CH], F16)
            d0c = singles.tile([128, CH], F16)
            ts8b = singles.tile([128, CH], F16)
            maskt = singles.tile([128, CH], F16)
            wt = singles.tile([128, KK, 2 * O], F16)
            b2 = singles.tile([128, 1], F32)
            zrow = singles.tile([1, 1312], F16)

            # ---------------- memsets (DVE; depth guards first so the sim
            # chain unblocks earliest)
            nc.vector.memset(zrow[:], 0.0)
            nc.vector.memset(d2f[0:2, 0 : 2 * W], 0.0)
            nc.vector.memset(d2f[0:2, Q1:FLATG], 0.0)
            nc.vector.memset(x2e[0:64, 0 : 2 * W], 0.0)
            nc.vector.memset(x2e[0:64, Q1 + W :], 0.0)
            nc.vector.memset(x2e[64:128, 0:W], 0.0)
            nc.vector.memset(x2e[64:128, Q1:], 0.0)

            # ---------------- depth chain (bf16; casting loads are SWDGE)
            hp = tc.high_priority()
            hp.__enter__()
            nc.gpsimd.dma_start(out=d2f[0:1, 2 * W : Q1 + W], in_=d_d[0:81, :])
            nc.gpsimd.dma_start(out=d2f[1:2, W:Q1], in_=d_d[79:160, :])
            nc.gpsimd.dma_start(out=maskt[:], in_=mask_d[:])
            nc.sync.dma_start(
                out=b2[:], in_=b_d.rearrange("(p one) -> p one", one=1)
            )
            # compact chunked dp: partition 16*(m+4h)+c <- d2f[h, SIM0+c*CH+off]
            for m, (dh, dw, off) in enumerate(MAPS):
                for h in range(2):
                    r = m + 4 * h
                    sl = d2f[h : h + 1, :]
                    src = bass.AP(
                        tensor=sl.tensor,
                        offset=sl.offset + SIM0 + off,
                        ap=[list(sl.ap)[0], [CH, NCHUNK], [1, CH]],
                    )
                    eng = nc.sync if r % 2 == 0 else nc.scalar
                    eng.dma_start(out=ts8c[16 * r : 16 * r + 16, :], in_=src)
            # d0 (map-independent; one 16-part group per (m, h))
            for m in range(4):
                for h in range(2):
                    r = m + 4 * h
                    sl = d2f[h : h + 1, :]
                    src = bass.AP(
                        tensor=sl.tensor,
                        offset=sl.offset + SIM0,
                        ap=[list(sl.ap)[0], [CH, NCHUNK], [1, CH]],
                    )
                    eng = nc.scalar if r % 2 == 0 else nc.sync
                    eng.dma_start(out=d0c[16 * r : 16 * r + 16, :], in_=src)

            # sim = exp(-(|dp - d0| + wrapmask)), wrapmask in {0, 50}
            nc.vector.tensor_sub(ts8c[:], ts8c[:], d0c[:])
            nc.scalar.activation(out=ts8c[:], in_=ts8c[:], func=Abs)
            nc.vector.tensor_add(ts8c[:], ts8c[:], maskt[:])
            nc.scalar.activation(
                out=ts8b[:], in_=ts8c[:], func=Exp, scale=-1.0
            )

            # chunk layout is row-flat: one DMA -> s8d [8, SROW] in DRAM
            # (row r at r*SROW, q-coords, zero head covers q in [0, 160))
            s8d = dramp.tile([8, SROW], F16)
            s8f = s8d[:]
            zdst = bass.AP(
                tensor=s8f.tensor,
                offset=s8f.offset,
                ap=[[SROW, 8], [1, SIM0]],
            )
            nc.sync.dma_start(out=zdst, in_=zrow[0:1, 0 : 8 * SIM0])
            mdst = bass.AP(
                tensor=s8f.tensor,
                offset=s8f.offset + SIM0,
                ap=[[SROW, 8], [CH, NCHUNK], [1, CH]],
            )
            nc.sync.dma_start(out=mdst, in_=ts8b[:])
            hp.__exit__(None, None, None)

            # ---------------- x loads (contiguous, casting SWDGE) + parity
            # 3 chunks x 27 rows per half; parity x2o[p] = x2e[p+1] follows
            # each chunk so early segments can start.  Modeled late
            # (wait_until) so the scheduler keeps the sim chain ahead of
            # them in the DMA rings.
            nc.gpsimd.dma_start(out=wt[:], in_=wt_d[:])
            with tc.tile_wait_until(0.03):
                for ci in range(3):
                    r0 = 27 * ci
                    nc.gpsimd.dma_start(
                        out=x2e[0:64, 2 * W + r0 * W : 2 * W + (r0 + 27) * W],
                        in_=x_d[:, r0 : r0 + 27, :].rearrange(
                            "c r w -> c (r w)"
                        ),
                    )
                    nc.gpsimd.dma_start(
                        out=x2e[64:128, W + r0 * W : W + (r0 + 27) * W],
                        in_=x_d[:, 79 + r0 : 79 + r0 + 27, :].rearrange(
                            "c r w -> c (r w)"
                        ),
                    )

            engs = {"sync": nc.sync, "scalar": nc.scalar, "gpsimd": nc.gpsimd}

            # per-segment odd-parity x spans, pre-issued so the loop has no
            # serial DMA-gen inside segments: x2os[s][i] = x2e[qs+1+i]
            x2os_all = []
            with tc.tile_wait_until(0.035):
                for s in range(NSEG):
                    qs = Q0 + s * SEGQ
                    x2os = x2op.tile([128, SEGQ + 164], F16, tag="x2o")
                    x2os_all.append(x2os)
                    nc.gpsimd.dma_start(
                        out=x2os[:, 0 : SEGQ + 160],
                        in_=x2e[:, qs + 1 : qs + 1 + SEGQ + 160],
                    )

            # ---------------- main loop (broadcasts prefetched 2 segs
            # ahead so the replicated DMAs overlap compute)
            PF = 0

            def issue_bcasts(s):
                qs = Q0 + s * SEGQ
                wa = qs - WINLO
                sims = []
                for m, (dh, dw, off) in enumerate(MAPS):
                    sim_m = simp.tile([128, WIN], F16, tag="sim")
                    sims.append(sim_m)
                    for h in range(2):
                        r = m + 4 * h
                        src = bass.AP(
                            tensor=s8f.tensor,
                            offset=s8f.offset + r * SROW + wa,
                            ap=[[0, 64], [1, WIN]],
                        )
                        engs[BCAST_ENG[(m, h)]].dma_start(
                            out=sim_m[64 * h : 64 * h + 64, :], in_=src
                        )
                return sims

            sims_q = [issue_bcasts(s) for s in range(min(PF + 1, NSEG))]
            for s in range(NSEG):
                qs = Q0 + s * SEGQ
                wa = qs - WINLO
                sims = sims_q[s]
                if s + PF + 1 < NSEG:
                    sims_q.append(issue_bcasts(s + PF + 1))

                x2os = x2os_all[s]

                # tight prods: prod_m covers [qs-off-d, +SEGQ+2d)
                prods = []
                pbase = []
                for m, (dh, dw, off) in enumerate(MAPS):
                    d = off % 2
                    pw = SEGQ + 2 * d
                    pr = prodp.tile([128, SEGQ + 2], F16, tag="prod")
                    prods.append(pr)
                    pbase.append(d)
                    ps0 = qs - off - d
                    nc.vector.tensor_mul(
                        pr[:, 0:pw],
                        x2e[:, ps0 : ps0 + pw],
                        sims[m][:, ps0 - wa : ps0 - wa + pw],
                    )

                stg = stgp.tile([128, SEGQ], F16, tag="stg")
                for j in range(NCH):
                    q = qs + j * CHW
                    so = WINLO + j * CHW           # even
                    psum = cpsum.tile([128, 1024], F32, tag="cps")
                    xms = []
                    for m, (dh, dw, off) in enumerate(MAPS):
                        xm = xmp.tile([128, CHW], F16, tag="xm")
                        xms.append(xm)
                        if off % 2:
                            xsrc = x2os[
                                :, off - 1 + j * CHW : off - 1 + j * CHW + CHW
                            ]
                        else:
                            xsrc = x2e[:, q + off : q + off + CHW]
                        eng = (
                            nc.gpsimd if (m, j) in POOL_XMS else nc.vector
                        )
                        eng.tensor_mul(
                            xm[:], xsrc, sims[m][:, so : so + CHW]
                        )
                    # taps-outer (consecutive matmuls share lhsT for
                    # ldw-opt); center + prod taps first, xm taps last.
                    taps = [(_tapidx(0, 0), x2e, q)]
                    for m, (dh, dw, off) in enumerate(MAPS):
                        taps.append(
                            (_tapidx(-dh, -dw), prods[m], pbase[m] + j * CHW)
                        )
                    for m, (dh, dw, off) in enumerate(MAPS):
                        taps.append((_tapidx(dh, dw), xms[m], 0))
                    for ti, (widx, rsrc, roff) in enumerate(taps):
                        for half in range(2):
                            pl, ph = 64 * half, 64 * half + 64
                            o2 = 0
                            for nn2 in SUBS:
                                nc.tensor.matmul(
                                    psum[pl:ph, o2 : o2 + nn2],
                                    wt[pl:ph, widx, 64 * half : 64 * half + 64],
                                    rsrc[pl:ph, roff + o2 : roff + o2 + nn2],
                                    start=(ti == 0),
                                    stop=(ti == len(taps) - 1),
                                    skip_group_check=True,
                                )
                                o2 += nn2
                    nc.scalar.activation(
                        out=stg[:, j * CHW : (j + 1) * CHW],
                        in_=psum[:, 0:CHW],
                        func=Ident,
                        bias=b2[:],
                        scale=1.0,
                    )

                of = out_d.rearrange("c h w -> c (h w)")
                nc.scalar.dma_start(
                    out=of[:, s * SEGQ : (s + 1) * SEGQ], in_=stg[0:64, :]
                )
                nc.scalar.dma_start(
                    out=of[:, NOUT + s * SEGQ : NOUT + (s + 1) * SEGQ],
                    in_=stg[64:128, :],
                )

    return nc


@functools.lru_cache(maxsize=1)
def _get_program():
    return _build_program()


def make_in_maps(x, depth, weights, bias):
    w_ckt = weights.reshape(O, C, KK).transpose(1, 2, 0)  # [C, KK, O]
    wt = np.zeros((2 * C, KK, 2 * O), np.float32)
    wt[0:C, :, 0:O] = w_ckt
    wt[C:, :, O:] = w_ckt
    wt = np.ascontiguousarray(wt)
    b2 = np.concatenate([bias, bias]).astype(np.float32)
    # wrap-around kill mask (additive, pre-exp) in the chunked compact-sim
    # layout: partition p = 16*(m+4h)+c, elem e <-> pixel q = SIM0+c*CH+e
    wmask = np.zeros((128, CH), np.float32)
    cols = np.arange(CH)
    for c in range(NCHUNK):
        colmod = (SIM0 + c * CH + cols) % W
        for m, col in MASK_COLS:
            for h in range(2):
                wmask[16 * (m + 4 * h) + c, colmod == col] = 50.0
    base = {"wt": wt, "bias2": b2, "wmask": wmask}
    return [
        {
            "x": np.ascontiguousarray(x[i]).astype(np.float32),
            "depth": np.ascontiguousarray(depth[i, 0]).astype(np.float32),
            **base,
        }
        for i in range(x.shape[0])
    ]


def kernel(x, depth, weights, bias):
    nc = _get_program()
    if not nc.is_finalized():
        nc.finalize()
    in_maps = make_in_maps(x, depth, weights, bias)
    res = run_bass_kernel_spmd(nc, in_maps, list(range(NCORES)))
    out = np.stack([np.asarray(res.results[i]["out"]) for i in range(NCORES)])
    return out.astype(np.float32)
